# revision 16
# baseline (speedup 1.0000x reference)
"""Trainium2 Bass kernel for nn_Backbone_36189394436309 (dense_mlp).

reference:
    x = tanh(LN(obs @ w1.T + b1) * g1 + be1)   obs [B,512] -> [B,128]
    x = tanh(LN(x @ w2.T + b2) * g2 + be2)     [B,128] -> [B,128]
    out = tanh(x @ w3.T + b3)                  [B,128] -> [B,128]

Strategy (pure data parallel over 8 cores, batch-sharded, feature-major):
  - bf16 I/O: host converts obs -> bf16 obsT [512,B]; device emits bf16
    outT [128,B]; host upcasts+transposes.  Halves the HBM traffic, which
    is the roofline for this shape (42 MB/core @ ~358 GB/s ~ 120 us).
  - LN mean-subtraction folds into centered weights host-side.  On-chip LN:
      c2   = (z+bc)^2                      (ACT Square | Pool copy + DVE mult)
      V_bc = (gamma*ones128)^T @ c2        (PE matmul -> variance, already
                                            broadcast across partitions)
      m    = NRSQ(V_bc)                    (one fused DVE op: quadratic seed
                                            + free-form Newton step, fitted
                                            per layer to sqrt(H)*rsqrt(V))
      xp   = (z + bc) * m                  (Pool scalar_tensor_tensor)
      x    = tanh(xp*g + be)               (ACT, 2048-wide over 4-tile groups)
  - Elementwise work is split across ACT/DVE/Pool so every engine stays
    under the DMA roofline; PE does 8 passes/tile (4x L1 K-chunks, 2 stats,
    L2, L3) at full bf16 column rate.
"""

import os
import sys
from contextlib import ExitStack

import numpy as np

for _p in ("/opt/trn_rl_repo", "/root/.axon_site/_ro/trn_rl_repo"):
    if os.path.isdir(_p) and _p not in sys.path:
        sys.path.insert(0, _p)

import concourse.bass as bass  # noqa: E402
import concourse.tile as tile  # noqa: E402
from concourse import bacc, mybir  # noqa: E402

F32 = mybir.dt.float32
BF16 = mybir.dt.bfloat16
ACT = mybir.ActivationFunctionType
ALU = mybir.AluOpType

EPS = 1e-5
N_CORES = 8
B_FULL = 262144
OBS = 512
H = 128
KC = OBS // 128
BLOC = B_FULL // N_CORES
NT = 512              # matmul / PSUM tile width (one PSUM bank)
OG = 4                # tiles per tanh/output group (2048-wide ACT)
NTILES = BLOC // NT   # 64
NGROUPS = NTILES // OG

# stats-matmul weight (exact bf16 power of two); the effective per-layer
# gamma is tuned continuously via a sqrt(sigma) fold into the squaring ops
# so the NR fit sits exactly at its flat (error-insensitive) cap point.
GAMMA_SB = 2.0 ** -10
# variance windows (relative to expected layer variance), chi^2-safe
WIN1 = (0.34, 2.20)
WIN2 = (0.32, 2.35)

# which tiles compute the layer-1 square directly on ACT (rest go through
# Pool copy + DVE square); load-balance knob
SQ1_ACT_FRAC = 0.5

_OPS = {}


def _register_ops():
    """Register the fused DVE micro-ops with the custom-DVE registry."""
    if _OPS:
        return _OPS
    from concourse import dve_ops
    from concourse.dve_spec import (
        C0,
        C1,
        C2,
        One,
        Spec,
        Src0,
        _has_src1,
        lower,
        sq,
    )
    from concourse.dve_uop import DveOpSpec

    # NRSQ: w = C0 + V*(C1 + V*C2); out = w*(1 - V*w^2)
    # fitted per layer so out ~ sqrt(H*gamma)*rsqrt(V + gamma*H*eps)
    _w = C0 + Src0 * (C1 + Src0 * C2)
    spec_nrsq = Spec(
        body=_w * (One - Src0 * sq(_w)),
        reference=lambda in0, in1, c0, c1, c2: (
            (lambda u: u * (1.0 - in0 * u * u))(c0 + in0 * (c1 + in0 * c2))
        ),
    )
    # SQB: out = (in0 + C0)^2   (C0 carries the per-partition folded bias)
    spec_sqb = Spec(
        body=sq(Src0 + C0),
        reference=lambda in0, in1, c0, c1, c2: (in0 + c0) ** 2,
    )
    for name, spec in (("ANT_NRSQ", spec_nrsq), ("ANT_SQB", spec_sqb)):
        if name in dve_ops._SUB_OPCODE_FOR_NAME:
            _OPS[name] = next(o for o in dve_ops.OPS if o.name == name)
            continue
        opcode = dve_ops._CUSTOM_DVE_ROW_BASE + len(dve_ops.OPS)
        dve_ops._SUB_OPCODE_FOR_NAME[name] = opcode
        shas = {}
        for ver in ("v3", "v4"):
            try:
                uops = lower(spec, ver=ver)
                shas[ver] = DveOpSpec(
                    name=name, opcode=opcode, uops=uops, rd1_en=_has_src1(spec)
                ).sha(ver)
            except Exception:
                pass
        op = dve_ops.DveOp(name, spec, subdim=False, uops_sha=shas)
        dve_ops.OPS.append(op)
        dve_ops.CUSTOM_DVE_SPECS[name] = spec
        _OPS[name] = op
    return _OPS


def fit_nrsq(gamma, s_l, lo, hi, n=4001):
    """Fit (c0,c1,c2) of F(V)=w(1-V w^2), w=c0+V(c1+V c2) to
    sqrt(H*gamma)/sqrt(V + gamma*H*EPS) over V in gamma*H*s_l*[lo,hi].
    Returns (c0, c1, c2, max_rel_err)."""
    x = np.geomspace(gamma * H * s_l * lo, gamma * H * s_l * hi, n)
    e = gamma * H * EPS
    g = np.sqrt(H * gamma) / np.sqrt(x + e)
    t_u = g / 2.0  # seed for the cubic solve of u(1-xu^2)=g is below cap
    # solve u*(1 - x u^2) = g for the small root via Newton
    u = t_u.copy()
    for _ in range(80):
        f = u * (1 - x * u * u) - g
        fp = 1 - 3 * x * u * u
        u = u - f / np.where(np.abs(fp) < 1e-30, 1e-30, fp)
    A = np.stack([np.ones_like(x), x, x * x], 1)
    c = np.linalg.lstsq(A, u, rcond=None)[0]

    def F_of(c):
        w = A @ c
        return w * (1.0 - x * w * w)

    def err(c):
        return float(np.max(np.abs(F_of(c) / g - 1.0)))

    best = (err(c), c.copy())
    lam = 1e-4
    for it in range(500):
        w = A @ c
        J = ((1.0 - 3.0 * x * w * w) / g)[:, None] * A
        r = F_of(c) / g - 1.0
        p = min(2 + it // 10 * 2, 64)
        wt = (np.abs(r) + 1e-16) ** ((p - 2) / 2.0)
        wt /= wt.max()
        Jw = J * wt[:, None]
        rw = r * wt
        M = Jw.T @ Jw + lam * np.diag(np.diag(Jw.T @ Jw) + 1e-30)
        try:
            d = np.linalg.solve(M, -(Jw.T @ rw))
        except np.linalg.LinAlgError:
            break
        c2_ = c + d
        e2 = err(c2_)
        if e2 < best[0]:
            best = (e2, c2_.copy())
            c = c2_
            lam = max(lam * 0.7, 1e-10)
        else:
            lam *= 3.0
            if lam > 1e8:
                break
    e_, c_ = best
    return float(c_[0]), float(c_[1]), float(c_[2]), e_


def expected_tanh_var(g, be):
    """E[feature variance] of tanh(g*u + be), u ~ N(0,1), Gauss-Hermite."""
    x, w = np.polynomial.hermite_e.hermegauss(101)
    w = w / w.sum()
    t = np.tanh(g[:, None] * x[None, :] + be[:, None])
    m1 = (t * w).sum(1)
    m2 = (t * t * w).sum(1)
    return float(m2.mean() - (m1.mean() ** 2))


def _bf16(x):
    import ml_dtypes

    return np.asarray(x).astype(ml_dtypes.bfloat16)


def fold_params(w1, b1, g1, be1, w2, b2, g2, be2, w3, b3):
    """Host-side folding; returns (const input map, per-layer NRSQ consts)."""
    f = np.float32

    def center(w, b):
        return (w - w.mean(axis=0, keepdims=True)).astype(f), (b - b.mean()).astype(f)

    w1c, b1c = center(w1, b1)
    w2c, b2c = center(w2, b2)

    s1 = float(np.mean(np.sum(w1c.astype(np.float64) ** 2, axis=1)))
    s2 = expected_tanh_var(g1.astype(np.float64), be1.astype(np.float64))
    s2 *= float(np.mean(np.sum(w2c.astype(np.float64) ** 2, axis=1)))
    s1 = max(s1, 1e-3)
    s2 = max(s2, 1e-3)

    gamma = float(np.float32(_bf16(GAMMA_SB)))  # exact bf16 value
    g_cap = 4.0 / (27.0 * H)
    fits = []
    for s_l, (lo, hi) in ((s1, WIN1), (s2, WIN2)):
        best = None
        for r in np.linspace(0.990, 1.000, 21):
            g_eff = float(r * g_cap)
            c0, c1, c2, e_ = fit_nrsq(g_eff, s_l, lo, hi)
            if best is None or e_ < best[3]:
                best = (c0, c1, c2, e_, g_eff)
        c0, c1, c2, e_, g_eff = best
        sq_scale = float(np.sqrt(g_eff / gamma))  # folded into the squaring op
        fits.append((c0, c1, c2, e_, sq_scale))

    consts = {
        "w1t": _bf16(w1c.T),                      # [512, 128]
        "w2t": _bf16(w2c.T),                      # [128, 128]
        "w3t": _bf16(w3.astype(f).T),             # [128, 128]
        "bc1": np.ascontiguousarray(b1c[:, None]),
        "bc2": np.ascontiguousarray(b2c[:, None]),
        "bcs1": np.ascontiguousarray((b1c * fits[0][4])[:, None]).astype(f),
        "bcs2": np.ascontiguousarray((b2c * fits[1][4])[:, None]).astype(f),
        # tanh scale with the 1/sqrt(sigma) unfold (z carries sqrt(sigma))
        "gs1": np.ascontiguousarray((g1.astype(f) / f(fits[0][4]))[:, None]),
        "gs2": np.ascontiguousarray((g2.astype(f) / f(fits[1][4]))[:, None]),
        "be1": np.ascontiguousarray(be1.astype(f)[:, None]),
        "be2": np.ascontiguousarray(be2.astype(f)[:, None]),
        "b3": np.ascontiguousarray(b3.astype(f)[:, None]),
    }
    return consts, gamma, fits


def declare_io(nc, bloc):
    t = {}
    t["obsT"] = nc.dram_tensor("obsT", [OBS, bloc], BF16, kind="ExternalInput").ap()
    t["w1t"] = nc.dram_tensor("w1t", [OBS, H], BF16, kind="ExternalInput").ap()
    t["w2t"] = nc.dram_tensor("w2t", [H, H], BF16, kind="ExternalInput").ap()
    t["w3t"] = nc.dram_tensor("w3t", [H, H], BF16, kind="ExternalInput").ap()
    for k in ("bc1", "bc2", "bcs1", "bcs2", "gs1", "gs2", "be1", "be2", "b3"):
        t[k] = nc.dram_tensor(k, [H, 1], F32, kind="ExternalInput").ap()
    t["outT"] = nc.dram_tensor("outT", [H, bloc], BF16, kind="ExternalOutput").ap()
    return t


def emit(ctx: ExitStack, tc: tile.TileContext, io, bloc, gamma, fits):
    nc = tc.nc
    ops = _register_ops()
    nrsq, sqb = ops["ANT_NRSQ"], ops["ANT_SQB"]

    consts = ctx.enter_context(tc.tile_pool(name="consts", bufs=1))
    xin = ctx.enter_context(tc.tile_pool(name="xin", bufs=3))
    work = ctx.enter_context(tc.tile_pool(name="work", bufs=3))
    xppool = ctx.enter_context(tc.tile_pool(name="xp", bufs=2))
    xpool = ctx.enter_context(tc.tile_pool(name="x", bufs=2))
    obuf = ctx.enter_context(tc.tile_pool(name="obuf", bufs=2))
    ps = ctx.enter_context(tc.tile_pool(name="ps", bufs=2, space="PSUM"))

    # --- constants ---
    w1t_sb = consts.tile([128, KC, H], BF16)
    nc.sync.dma_start(w1t_sb[:], io["w1t"].rearrange("(c p) m -> p c m", p=128))
    w2t_sb = consts.tile([128, H], BF16)
    nc.sync.dma_start(w2t_sb[:], io["w2t"])
    w3t_sb = consts.tile([128, H], BF16)
    nc.sync.dma_start(w3t_sb[:], io["w3t"])
    small = {}
    for k in ("bc1", "bc2", "bcs1", "bcs2", "gs1", "gs2", "be1", "be2", "b3"):
        small[k] = consts.tile([128, 1], F32, name=f"sm_{k}", tag=f"sm_{k}")
        nc.sync.dma_start(small[k][:], io[k])
    ones_g = consts.tile([128, H], BF16, name="ones_g", tag="ones_g")
    nc.vector.memset(ones_g[:], gamma)

    GP = 2  # tiles per input DMA
    # pipeline state, keyed by tile index or group index
    xts = {}          # load-group -> input tile
    d1s, d2s, d3s = {}, {}, {}
    zs = {}           # (tile, layer) -> scaled pre-LN tile (bf16, SBUF)
    c2s = {}          # (tile, layer) -> squared tile
    vs = {}           # (tile, layer) -> variance psum
    ms = {}           # (tile, layer) -> m tile
    xpg = {}          # (group, layer) -> xp group buffer
    xg = {}           # (group, layer) -> x group buffer (bf16)
    obs_ = {}         # group -> output buffer

    nzx_act = int(round(OG * SQ1_ACT_FRAC))

    def load(g):
        if not (0 <= g < NTILES // GP):
            return
        xt = xin.tile([128, KC, GP * NT], BF16, name=f"xt{g}", tag="xt")
        b0 = g * GP * NT
        nc.sync.dma_start(
            xt[:],
            io["obsT"][:, b0 : b0 + GP * NT].rearrange("(c p) n -> p c n", p=128),
        )
        xts[g] = xt

    def l1mm(j):
        if not (0 <= j < NTILES):
            return
        g, h = divmod(j, GP)
        xt = xts[g]
        d1 = ps.tile([128, NT], F32, name=f"d1_{j}", tag="d", bufs=5)
        for c in range(KC):
            nc.tensor.matmul(
                d1[:],
                w1t_sb[:, c, :],
                xt[:, c, h * NT : (h + 1) * NT],
                start=(c == 0),
                stop=(c == KC - 1),
            )
        d1s[j] = d1
        if h == GP - 1:
            del xts[g]

    def zx(j, layer):
        """z = (d + bc)*sqrt(sigma) -> SBUF bf16 (the only PSUM read of d).
        Engine split ACT/DVE by tile index."""
        if not (0 <= j < NTILES):
            return
        d = (d1s if layer == 0 else d2s).pop(j)
        ss = fits[layer][4]
        z = work.tile([128, NT], BF16, tag="z", bufs=8)
        if (j % OG) < nzx_act:
            bcs = small["bcs1" if layer == 0 else "bcs2"]
            nc.scalar.activation(z[:], d[:], ACT.Identity, bias=bcs[:], scale=ss)
        else:
            bc = small["bc1" if layer == 0 else "bc2"]
            nc.vector.tensor_scalar(z[:], d[:], bc[:], ss, ALU.add, ALU.mult)
        zs[(j, layer)] = z

    def sq(j, layer):
        """c2 = z*z on Pool (SBUF-only engine)."""
        if not (0 <= j < NTILES):
            return
        z = zs[(j, layer)]
        c2 = work.tile([128, NT], BF16, tag="c2", bufs=6)
        nc.gpsimd.tensor_tensor(c2[:], z[:], z[:], ALU.mult)
        c2s[(j, layer)] = c2

    def vmm(j, layer):
        if not (0 <= j < NTILES):
            return
        c2 = c2s.pop((j, layer))
        v = ps.tile([128, NT], F32, tag="v", bufs=3)
        nc.tensor.matmul(v[:], ones_g[:], c2[:], start=True, stop=True)
        vs[(j, layer)] = v

    def rsq(j, layer):
        if not (0 <= j < NTILES):
            return
        v = vs.pop((j, layer))
        c0, c1, c2_ = fits[layer][:3]
        m = work.tile([128, NT], BF16, tag="m", bufs=6)
        nc.vector._custom_dve(nrsq, out=m[:], in0=v[:], s0=c0, s1=c1, imm2=c2_)
        ms[(j, layer)] = m

    def apply_(j, layer):
        """xp[:, j%OG, :] = z * m  on Pool (SBUF only)."""
        if not (0 <= j < NTILES):
            return
        g = j // OG
        if (g, layer) not in xpg:
            xpg[(g, layer)] = xppool.tile(
                [128, OG, NT], F32, name=f"xp{layer}_{g}", tag=f"xp{layer}"
            )
        xp = xpg[(g, layer)]
        z = zs.pop((j, layer))
        m = ms.pop((j, layer))
        nc.gpsimd.tensor_tensor(xp[:, j % OG, :], z[:], m[:], ALU.mult)

    def tanh_group(g, layer):
        """x = tanh(xp*(g/sqrt(sigma)) + be) over the group (2048-wide)."""
        if not (0 <= g < NGROUPS):
            return
        xp = xpg.pop((g, layer))
        x = xpool.tile([128, OG, NT], BF16, name=f"x{layer}_{g}", tag=f"x{layer}")
        g_sb = small["gs1" if layer == 0 else "gs2"]
        be_sb = small["be1" if layer == 0 else "be2"]
        nc.scalar.activation(
            x[:].rearrange("p g n -> p (g n)"),
            xp[:].rearrange("p g n -> p (g n)"),
            ACT.Tanh,
            bias=be_sb[:],
            scale=g_sb[:],
        )
        xg[(g, layer)] = x

    def l2mm(j):
        if not (0 <= j < NTILES):
            return
        g = j // OG
        x = xg[(g, 0)]
        d2 = ps.tile([128, NT], F32, name=f"d2_{j}", tag="d", bufs=5)
        nc.tensor.matmul(d2[:], w2t_sb[:], x[:, j % OG, :], start=True, stop=True)
        d2s[j] = d2
        if j % OG == OG - 1:
            del xg[(g, 0)]

    def l3mm(j):
        if not (0 <= j < NTILES):
            return
        g = j // OG
        x = xg[(g, 1)]
        d3 = ps.tile([128, NT], F32, name=f"d3_{j}", tag="d", bufs=5)
        nc.tensor.matmul(d3[:], w3t_sb[:], x[:, j % OG, :], start=True, stop=True)
        d3s[j] = d3
        if j % OG == OG - 1:
            del xg[(g, 1)]

    def tail(j):
        """ob[:, j%OG, :] = tanh(d3 + b3); DMA out when the group fills."""
        if not (0 <= j < NTILES):
            return
        g = j // OG
        if g not in obs_:
            obs_[g] = obuf.tile([128, OG, NT], BF16, name=f"ob{g}", tag="ob")
        ob = obs_[g]
        d3 = d3s.pop(j)
        nc.scalar.activation(ob[:, j % OG, :], d3[:], ACT.Tanh, bias=small["b3"][:])
        if j % OG == OG - 1:
            b0 = g * OG * NT
            nc.sync.dma_start(
                io["outT"][:, b0 : b0 + OG * NT],
                obs_.pop(g)[:].rearrange("p g n -> p (g n)"),
            )

    # --- software pipeline over 4-tile groups, skewed by layer ---
    # Group s runs layer-1 LN; group s-1 runs layer-2; group s-2 runs
    # layer-3 + store.  Within a step, per-tile ops are interleaved so each
    # engine queue always has ready work.
    def ln_chain(j, layer):
        zx(j, layer)
        sq(j, layer)
        vmm(j, layer)
        rsq(j, layer)
        apply_(j, layer)

    load(0)
    load(1)
    for j in range(OG):
        l1mm(j)
    for s in range(NGROUPS + 3):
        base = s * OG
        # prefetch next group's input while this step computes
        load(s * 2 + 2)
        load(s * 2 + 3)
        for k in range(OG):
            # layer-1 LN for group s
            ln_chain(base + k, 0)
            # layer-1 matmuls for group s+1 (keeps PE fed)
            l1mm(base + OG + k)
            # layer-2 for group s-1
            j2 = base - OG + k
            if k == 0:
                tanh_group(s - 1, 0)
            l2mm(j2)
            ln_chain(j2, 1)
            # layer-3 for group s-2
            j3 = base - 2 * OG + k
            if k == 0:
                tanh_group(s - 2, 1)
            l3mm(j3)
            tail(j3)


def build_program(bloc, gamma, fits):
    nc = bacc.Bacc(
        "TRN2",
        target_bir_lowering=False,
        debug=False,
        enable_asserts=False,
        num_devices=1,
    )
    io = declare_io(nc, bloc)
    with tile.TileContext(nc) as tc:
        with ExitStack() as ctx:
            emit(ctx, tc, io, bloc, gamma, fits)
    nc.compile()
    return nc


def kernel(**inputs):
    from concourse.bass_utils import run_bass_kernel_spmd

    obs = np.asarray(inputs["obs"], dtype=np.float32)
    consts, gamma, fits = fold_params(
        *[
            np.asarray(inputs[k], dtype=np.float32)
            for k in ("w1", "b1", "g1", "be1", "w2", "b2", "g2", "be2", "w3", "b3")
        ]
    )
    obsT = np.ascontiguousarray(_bf16(obs).T)  # [512, B] bf16

    nc = build_program(BLOC, gamma, fits)
    in_maps = []
    for c in range(N_CORES):
        m = {"obsT": np.ascontiguousarray(obsT[:, c * BLOC : (c + 1) * BLOC])}
        m.update(consts)
        in_maps.append(m)
    res = run_bass_kernel_spmd(nc, in_maps, core_ids=list(range(N_CORES)))
    global LAST_RESULTS
    LAST_RESULTS = res
    out = np.empty((B_FULL, H), dtype=np.float32)
    for c in range(N_CORES):
        out[c * BLOC : (c + 1) * BLOC] = res.results[c]["outT"].astype(np.float32).T
    return out


LAST_RESULTS = None


# revision 19
# speedup vs baseline: 1.3958x; 1.3958x over previous
"""Trainium2 Bass kernel for nn_Backbone_36189394436309 (dense_mlp).

reference:
    x = tanh(LN(obs @ w1.T + b1) * g1 + be1)   obs [B,512] -> [B,128]
    x = tanh(LN(x @ w2.T + b2) * g2 + be2)     [B,128] -> [B,128]
    out = tanh(x @ w3.T + b3)                  [B,128] -> [B,128]

Strategy (pure data parallel over 8 cores, batch-sharded, feature-major):
  - bf16 input / bf16 output DMA (obsT [512,B] bf16, outT [128,B] bf16):
    halves HBM traffic, the roofline for this shape (~42 MB/core).
  - L1 matmul in bf16 (obs is bf16 anyway); L2/L3 in float32r (same PE
    column rate at N=512, removes weight-quantization error).
  - LN mean-centering folds into the weights host-side.  Per layer:
      z    = (d + bc)*sqrt(sigma)          (zx: ACT Identity | DVE ts, the
                                            only PSUM read of d, 1024-wide)
      c2   = z*z                           (Pool | DVE tensor_tensor, bf16)
      V_bc = (gamma*ones128)^T @ c2        (PE -> broadcast variance, PSUM)
      m    = NRSQ(V_bc)                    (one fused DVE op: quadratic seed
                                            + Newton step at its flat point,
                                            fitted per layer, 1024-wide)
      xp   = z*m                           (DVE bf16 2x/4x mode | Pool)
      x    = tanh(xp*(g/sqrt(sigma)) + be) (ACT, 2048-wide, f32 out)
  - All PSUM-reading passes run 1024-wide over tile pairs; PSUM holds
    d1-pairs+d3-pairs (4 banks), d2-pairs (2), v-pairs (2).
"""

import os
import sys
from contextlib import ExitStack

import numpy as np

for _p in ("/opt/trn_rl_repo", "/root/.axon_site/_ro/trn_rl_repo"):
    if os.path.isdir(_p) and _p not in sys.path:
        sys.path.insert(0, _p)

import concourse.bass as bass  # noqa: E402
import concourse.tile as tile  # noqa: E402
from concourse import bacc, mybir  # noqa: E402

F32 = mybir.dt.float32
F32R = mybir.dt.float32r
BF16 = mybir.dt.bfloat16
ACT = mybir.ActivationFunctionType
ALU = mybir.AluOpType

EPS = 1e-5
N_CORES = 8
B_FULL = 262144
OBS = 512
H = 128
KC = OBS // 128
BLOC = B_FULL // N_CORES
NT = 512              # matmul / PSUM bank width
PG = 2                # tiles per PSUM pair (1024-wide elementwise passes)
OG = 4                # tiles per tanh/output group (2048-wide ACT)
NTILES = BLOC // NT   # 64
NPAIRS = NTILES // PG
NGROUPS = NTILES // OG

# stats-matmul weight (exact bf16 power of two); the effective per-layer
# gamma is tuned continuously via a sqrt(sigma) fold into the zx pass.
GAMMA_SB = 2.0 ** -10
# variance windows (relative to expected layer variance).  Layer-2 variance
# is LN-constrained and concentrates hard; windows carry >=25% headroom
# over the observed full-batch range.
WIN1 = (0.40, 1.90)
WIN2 = (0.70, 1.32)

_OPS = {}


def _register_ops():
    if _OPS:
        return _OPS
    from concourse import dve_ops
    from concourse.dve_spec import C0, C1, C2, One, Spec, Src0, _has_src1, lower, sq
    from concourse.dve_uop import DveOpSpec

    # NRSQ: w = C0 + V*(C1 + V*C2); out = w*(1 - V*w^2)
    # fitted per layer so out ~ sqrt(H*gamma_eff)*rsqrt(V + gamma_eff*H*eps)
    _w = C0 + Src0 * (C1 + Src0 * C2)
    spec_nrsq = Spec(
        body=_w * (One - Src0 * sq(_w)),
        reference=lambda in0, in1, c0, c1, c2: (
            (lambda u: u * (1.0 - in0 * u * u))(c0 + in0 * (c1 + in0 * c2))
        ),
    )
    for name, spec in (("ANT_NRSQ", spec_nrsq),):
        if name in dve_ops._SUB_OPCODE_FOR_NAME:
            _OPS[name] = next(o for o in dve_ops.OPS if o.name == name)
            continue
        opcode = dve_ops._CUSTOM_DVE_ROW_BASE + len(dve_ops.OPS)
        dve_ops._SUB_OPCODE_FOR_NAME[name] = opcode
        shas = {}
        for ver in ("v3", "v4"):
            try:
                uops = lower(spec, ver=ver)
                shas[ver] = DveOpSpec(
                    name=name, opcode=opcode, uops=uops, rd1_en=_has_src1(spec)
                ).sha(ver)
            except Exception:
                pass
        op = dve_ops.DveOp(name, spec, subdim=False, uops_sha=shas)
        dve_ops.OPS.append(op)
        dve_ops.CUSTOM_DVE_SPECS[name] = spec
        _OPS[name] = op
    return _OPS


def fit_nrsq(gamma, s_l, lo, hi, n=4001):
    """Fit (c0,c1,c2) of F(V)=w(1-V w^2), w=c0+V(c1+V c2) to
    sqrt(H*gamma)/sqrt(V + gamma*H*EPS) over V in gamma*H*s_l*[lo,hi]."""
    x = np.geomspace(gamma * H * s_l * lo, gamma * H * s_l * hi, n)
    e = gamma * H * EPS
    g = np.sqrt(H * gamma) / np.sqrt(x + e)
    u = g / 2.0
    for _ in range(80):
        f = u * (1 - x * u * u) - g
        fp = 1 - 3 * x * u * u
        u = u - f / np.where(np.abs(fp) < 1e-30, 1e-30, fp)
    A = np.stack([np.ones_like(x), x, x * x], 1)
    c = np.linalg.lstsq(A, u, rcond=None)[0]

    def F_of(c):
        w = A @ c
        return w * (1.0 - x * w * w)

    def err(c):
        return float(np.max(np.abs(F_of(c) / g - 1.0)))

    best = (err(c), c.copy())
    lam = 1e-4
    for it in range(500):
        w = A @ c
        J = ((1.0 - 3.0 * x * w * w) / g)[:, None] * A
        r = F_of(c) / g - 1.0
        p = min(2 + it // 10 * 2, 64)
        wt = (np.abs(r) + 1e-16) ** ((p - 2) / 2.0)
        wt /= wt.max()
        Jw = J * wt[:, None]
        rw = r * wt
        M = Jw.T @ Jw + lam * np.diag(np.diag(Jw.T @ Jw) + 1e-30)
        try:
            d = np.linalg.solve(M, -(Jw.T @ rw))
        except np.linalg.LinAlgError:
            break
        c2_ = c + d
        e2 = err(c2_)
        if e2 < best[0]:
            best = (e2, c2_.copy())
            c = c2_
            lam = max(lam * 0.7, 1e-10)
        else:
            lam *= 3.0
            if lam > 1e8:
                break
    e_, c_ = best
    return float(c_[0]), float(c_[1]), float(c_[2]), e_


def expected_tanh_var(g, be):
    x, w = np.polynomial.hermite_e.hermegauss(101)
    w = w / w.sum()
    t = np.tanh(g[:, None] * x[None, :] + be[:, None])
    m1 = (t * w).sum(1)
    m2 = (t * t * w).sum(1)
    return float(m2.mean() - (m1.mean() ** 2))


def _bf16(x):
    import ml_dtypes

    return np.asarray(x).astype(ml_dtypes.bfloat16)


def fold_params(w1, b1, g1, be1, w2, b2, g2, be2, w3, b3):
    f = np.float32

    def center(w, b):
        return (w - w.mean(axis=0, keepdims=True)).astype(f), (b - b.mean()).astype(f)

    w1c, b1c = center(w1, b1)
    w2c, b2c = center(w2, b2)

    s1 = float(np.mean(np.sum(w1c.astype(np.float64) ** 2, axis=1)))
    s2 = expected_tanh_var(g1.astype(np.float64), be1.astype(np.float64))
    s2 *= float(np.mean(np.sum(w2c.astype(np.float64) ** 2, axis=1)))
    s1 = max(s1, 1e-3)
    s2 = max(s2, 1e-3)

    gamma = float(np.float32(_bf16(GAMMA_SB)))
    g_cap = 4.0 / (27.0 * H)
    fits = []
    for s_l, (lo, hi) in ((s1, WIN1), (s2, WIN2)):
        best = None
        for r in np.linspace(0.990, 1.000, 21):
            g_eff = float(r * g_cap)
            c0, c1, c2, e_ = fit_nrsq(g_eff, s_l, lo, hi)
            if best is None or e_ < best[3]:
                best = (c0, c1, c2, e_, g_eff)
        c0, c1, c2, e_, g_eff = best
        sq_scale = float(np.sqrt(g_eff / gamma))
        fits.append((c0, c1, c2, e_, sq_scale))

    consts = {
        "w1t": _bf16(w1c.T),                                  # [512,128] bf16
        "w2t": np.ascontiguousarray(w2c.T),                   # [128,128] f32r
        "w3t": np.ascontiguousarray(w3.astype(f).T),          # [128,128] f32r
        "bc1": np.ascontiguousarray(b1c[:, None]),
        "bc2": np.ascontiguousarray(b2c[:, None]),
        "bcs1": np.ascontiguousarray((b1c * fits[0][4])[:, None]).astype(f),
        "bcs2": np.ascontiguousarray((b2c * fits[1][4])[:, None]).astype(f),
        "gs1": np.ascontiguousarray((g1.astype(f) / f(fits[0][4]))[:, None]),
        "gs2": np.ascontiguousarray((g2.astype(f) / f(fits[1][4]))[:, None]),
        "be1": np.ascontiguousarray(be1.astype(f)[:, None]),
        "be2": np.ascontiguousarray(be2.astype(f)[:, None]),
        "b3": np.ascontiguousarray(b3.astype(f)[:, None]),
    }
    return consts, gamma, fits


def declare_io(nc, bloc):
    t = {}
    t["obsT"] = nc.dram_tensor("obsT", [OBS, bloc], BF16, kind="ExternalInput").ap()
    t["w1t"] = nc.dram_tensor("w1t", [OBS, H], BF16, kind="ExternalInput").ap()
    t["w2t"] = nc.dram_tensor("w2t", [H, H], F32R, kind="ExternalInput").ap()
    t["w3t"] = nc.dram_tensor("w3t", [H, H], F32R, kind="ExternalInput").ap()
    for k in ("bc1", "bc2", "bcs1", "bcs2", "gs1", "gs2", "be1", "be2", "b3"):
        t[k] = nc.dram_tensor(k, [H, 1], F32, kind="ExternalInput").ap()
    t["outT"] = nc.dram_tensor("outT", [H, bloc], BF16, kind="ExternalOutput").ap()
    return t


def emit(ctx: ExitStack, tc: tile.TileContext, io, bloc, gamma, fits):
    nc = tc.nc
    ops = _register_ops()
    nrsq = ops["ANT_NRSQ"]

    consts = ctx.enter_context(tc.tile_pool(name="consts", bufs=1))
    xin = ctx.enter_context(tc.tile_pool(name="xin", bufs=3))
    work = ctx.enter_context(tc.tile_pool(name="work", bufs=3))
    xppool = ctx.enter_context(tc.tile_pool(name="xp", bufs=2))
    xpool = ctx.enter_context(tc.tile_pool(name="x", bufs=2))
    obuf = ctx.enter_context(tc.tile_pool(name="obuf", bufs=2))
    ps = ctx.enter_context(tc.tile_pool(name="ps", bufs=1, space="PSUM"))

    # --- constants ---
    w1t_sb = consts.tile([128, KC, H], BF16)
    nc.sync.dma_start(w1t_sb[:], io["w1t"].rearrange("(c p) m -> p c m", p=128))
    w2t_sb = consts.tile([128, H], F32R)
    nc.sync.dma_start(w2t_sb[:], io["w2t"])
    w3t_sb = consts.tile([128, H], F32R)
    nc.sync.dma_start(w3t_sb[:], io["w3t"])
    w2r, w3r = w2t_sb[:], w3t_sb[:]
    small = {}
    for k in ("bc1", "bc2", "bcs1", "bcs2", "gs1", "gs2", "be1", "be2", "b3"):
        small[k] = consts.tile([128, 1], F32, name=f"sm_{k}", tag=f"sm_{k}")
        nc.sync.dma_start(small[k][:], io[k])
    ones_g = consts.tile([128, H], BF16, name="ones_g", tag="ones_g")
    nc.vector.memset(ones_g[:], gamma)

    # pipeline state, keyed by pair index p (pair = tiles 2p, 2p+1)
    xts = {}
    d1s, d2s, d3s = {}, {}, {}
    zs = {}           # (pair, layer) -> z pair tile [128,2,NT] bf16
    c2s = {}          # (pair, layer)
    vps = {}          # (pair, layer) -> v pair psum
    ms = {}           # (pair, layer)
    xpg = {}          # (group, layer) -> xp group buffer bf16
    xg = {}           # (group, layer) -> x group buffer f32
    obs_ = {}         # group -> output buffer

    def load(p):
        if not (0 <= p < NPAIRS):
            return
        xt = xin.tile([128, KC, PG * NT], BF16, name=f"xt{p}", tag="xt")
        b0 = p * PG * NT
        nc.sync.dma_start(
            xt[:],
            io["obsT"][:, b0 : b0 + PG * NT].rearrange("(c p) n -> p c n", p=128),
        )
        xts[p] = xt

    def l1mm(p):
        if not (0 <= p < NPAIRS):
            return
        xt = xts.pop(p)
        d1 = ps.tile([128, PG, NT], F32, name=f"d1_{p}", tag="dp", bufs=2)
        for c in range(KC):
            for h in range(PG):
                nc.tensor.matmul(
                    d1[:, h, :],
                    w1t_sb[:, c, :],
                    xt[:, c, h * NT : (h + 1) * NT],
                    start=(c == 0),
                    stop=(c == KC - 1),
                )
        d1s[p] = d1

    def zx(p, layer):
        """z = (d + bc)*sqrt(sigma) -> SBUF bf16, 1024-wide. ACT/DVE split."""
        if not (0 <= p < NPAIRS):
            return
        d = (d1s if layer == 0 else d2s).pop(p)
        ss = fits[layer][4]
        z = work.tile([128, PG, NT], BF16, tag="z", bufs=6)
        dw = d[:].rearrange("p g n -> p (g n)")
        zw = z[:].rearrange("p g n -> p (g n)")
        if (p + layer) % 3 != 0:
            bcs = small["bcs1" if layer == 0 else "bcs2"]
            nc.scalar.activation(zw, dw, ACT.Identity, bias=bcs[:], scale=ss)
        else:
            bc = small["bc1" if layer == 0 else "bc2"]
            nc.vector.tensor_scalar(zw, dw, bc[:], ss, ALU.add, ALU.mult)
        zs[(p, layer)] = z

    def sq(p, layer):
        """c2 = z*z (SBUF bf16), Pool with a small DVE share."""
        if not (0 <= p < NPAIRS):
            return
        z = zs[(p, layer)]
        c2 = work.tile([128, PG, NT], BF16, tag="c2", bufs=4)
        zw = z[:].rearrange("p g n -> p (g n)")
        cw = c2[:].rearrange("p g n -> p (g n)")
        if (p + layer) % 8 == 7:
            nc.vector.tensor_tensor(cw, zw, zw, ALU.mult)
        else:
            nc.gpsimd.tensor_tensor(cw, zw, zw, ALU.mult)
        c2s[(p, layer)] = c2

    def vmm(p, layer):
        if not (0 <= p < NPAIRS):
            return
        c2 = c2s.pop((p, layer))
        v = ps.tile([128, PG, NT], F32, tag="vp", bufs=1)
        for h in range(PG):
            nc.tensor.matmul(v[:, h, :], ones_g[:], c2[:, h, :], start=True, stop=True)
        vps[(p, layer)] = v

    def rsq(p, layer):
        if not (0 <= p < NPAIRS):
            return
        v = vps.pop((p, layer))
        c0, c1, c2_ = fits[layer][:3]
        m = work.tile([128, PG, NT], BF16, tag="m", bufs=4)
        nc.vector._custom_dve(
            nrsq,
            out=m[:].rearrange("p g n -> p (g n)"),
            in0=v[:].rearrange("p g n -> p (g n)"),
            s0=c0,
            s1=c1,
            imm2=c2_,
        )
        ms[(p, layer)] = m

    def apply_(p, layer):
        """xp pair-slice = z*m on DVE (bf16 fast mode)."""
        if not (0 <= p < NPAIRS):
            return
        g = p // (OG // PG)
        if (g, layer) not in xpg:
            xpg[(g, layer)] = xppool.tile(
                [128, OG, NT], BF16, name=f"xp{layer}_{g}", tag=f"xp{layer}"
            )
        xp = xpg[(g, layer)]
        z = zs.pop((p, layer))
        m = ms.pop((p, layer))
        s0 = (p % (OG // PG)) * PG
        nc.vector.tensor_tensor(
            xp[:, s0 : s0 + PG, :].rearrange("p g n -> p (g n)"),
            z[:].rearrange("p g n -> p (g n)"),
            m[:].rearrange("p g n -> p (g n)"),
            ALU.mult,
        )

    def tanh_group(g, layer):
        """x = tanh(xp*(g/sqrt(sigma)) + be), 2048-wide, f32 out."""
        if not (0 <= g < NGROUPS):
            return
        xp = xpg.pop((g, layer))
        x = xpool.tile([128, OG, NT], F32R, name=f"x{layer}_{g}", tag=f"x{layer}")
        g_sb = small["gs1" if layer == 0 else "gs2"]
        be_sb = small["be1" if layer == 0 else "be2"]
        nc.scalar.activation(
            x[:].rearrange("p g n -> p (g n)"),
            xp[:].rearrange("p g n -> p (g n)"),
            ACT.Tanh,
            bias=be_sb[:],
            scale=g_sb[:],
        )
        xg[(g, layer)] = x

    def l2mm(p):
        if not (0 <= p < NPAIRS):
            return
        g = p // (OG // PG)
        x = xg[(g, 0)]
        d2 = ps.tile([128, PG, NT], F32, name=f"d2_{p}", tag="d2p", bufs=1)
        s0 = (p % (OG // PG)) * PG
        for h in range(PG):
            nc.tensor.matmul(
                d2[:, h, :], w2r, x[:, s0 + h, :], start=True, stop=True
            )
        d2s[p] = d2
        if s0 + PG == OG:
            del xg[(g, 0)]

    def l3mm(p):
        if not (0 <= p < NPAIRS):
            return
        g = p // (OG // PG)
        x = xg[(g, 1)]
        d3 = ps.tile([128, PG, NT], F32, name=f"d3_{p}", tag="dp", bufs=2)
        s0 = (p % (OG // PG)) * PG
        for h in range(PG):
            nc.tensor.matmul(
                d3[:, h, :], w3r, x[:, s0 + h, :], start=True, stop=True
            )
        d3s[p] = d3
        if s0 + PG == OG:
            del xg[(g, 1)]

    def tail(p):
        """ob pair-slice = tanh(d3 + b3), 1024-wide; DMA out per group."""
        if not (0 <= p < NPAIRS):
            return
        g = p // (OG // PG)
        if g not in obs_:
            obs_[g] = obuf.tile([128, OG, NT], BF16, name=f"ob{g}", tag="ob")
        ob = obs_[g]
        d3 = d3s.pop(p)
        s0 = (p % (OG // PG)) * PG
        nc.scalar.activation(
            ob[:, s0 : s0 + PG, :].rearrange("p g n -> p (g n)"),
            d3[:].rearrange("p g n -> p (g n)"),
            ACT.Tanh,
            bias=small["b3"][:],
        )
        if s0 + PG == OG:
            b0 = g * OG * NT
            nc.sync.dma_start(
                io["outT"][:, b0 : b0 + OG * NT],
                obs_.pop(g)[:].rearrange("p g n -> p (g n)"),
            )

    # --- software pipeline over pairs, skewed by layer ---
    # Emission order within a step is engine-queue order; it is arranged so
    # PE's in-order queue never parks on a PSUM-slot wait before independent
    # matmuls, and each engine has ready work from an adjacent stage.
    GPP = OG // PG  # pairs per tanh group (2)
    load(0)
    load(1)
    l1mm(0)
    for s in range(NPAIRS + 2 * GPP + 2):
        load(s + 2)
        p2 = s - GPP
        p3 = s - 2 * GPP
        # layer-1 LN front half for pair s; frees d1(s) early
        zx(s, 0)
        sq(s, 0)
        l1mm(s + 1)
        vmm(s, 0)
        rsq(s, 0)
        apply_(s, 0)
        # layer-2 for the pair one tanh-group behind
        if 0 <= p2 and p2 % GPP == 0:
            tanh_group(p2 // GPP, 0)
        l2mm(p2)
        zx(p2, 1)
        sq(p2, 1)
        vmm(p2, 1)
        rsq(p2, 1)
        apply_(p2, 1)
        # layer-3 one more group behind
        if 0 <= p3 and p3 % GPP == 0:
            tanh_group(p3 // GPP, 1)
        l3mm(p3)
        tail(p3)


def build_program(bloc, gamma, fits):
    nc = bacc.Bacc(
        "TRN2",
        target_bir_lowering=False,
        debug=False,
        enable_asserts=False,
        num_devices=1,
    )
    io = declare_io(nc, bloc)
    with tile.TileContext(nc) as tc:
        with ExitStack() as ctx:
            emit(ctx, tc, io, bloc, gamma, fits)
    nc.compile()
    return nc


def kernel(**inputs):
    from concourse.bass_utils import run_bass_kernel_spmd

    obs = np.asarray(inputs["obs"], dtype=np.float32)
    consts, gamma, fits = fold_params(
        *[
            np.asarray(inputs[k], dtype=np.float32)
            for k in ("w1", "b1", "g1", "be1", "w2", "b2", "g2", "be2", "w3", "b3")
        ]
    )
    obsT = np.ascontiguousarray(_bf16(obs).T)  # [512, B] bf16

    nc = build_program(BLOC, gamma, fits)
    in_maps = []
    for c in range(N_CORES):
        m = {"obsT": np.ascontiguousarray(obsT[:, c * BLOC : (c + 1) * BLOC])}
        m.update(consts)
        in_maps.append(m)
    res = run_bass_kernel_spmd(nc, in_maps, core_ids=list(range(N_CORES)))
    global LAST_RESULTS
    LAST_RESULTS = res
    out = np.empty((B_FULL, H), dtype=np.float32)
    for c in range(N_CORES):
        out[c * BLOC : (c + 1) * BLOC] = res.results[c]["outT"].astype(np.float32).T
    return out


LAST_RESULTS = None
